# revision 15
# baseline (speedup 1.0000x reference)
"""DecorrLoss kernel for 8 trn2 NeuronCores.

Math (matches reference DecorrLoss.forward):
  x: (8, 4096, 512) fp32, flattened to N=32768 samples of d=512.
  G  = X^T X            (512 x 512 Gram, summed over all samples)
  S1 = sum x^2  = trace(G)
  S2 = sum x^4
  S3 = sum_n (||x_n||^2)^2
  grad             = (1-kappa) * offdiag(G/N) + kappa * (diag(G/N) - I)
  correlation_loss = (S3 - S2) / (N d^2)
  whitening_loss   = (S2 - 2 S1 + N d) / (N d^2)

Sharding: data-parallel over the batch axis -- core c processes x[c]
(4096 x 512), producing a partial Gram (block-upper triangle only; the
host mirrors it) and per-partition partial sums for S2/S3.  The host
all-reduces the 8 partials in float64 and applies the kappa formula.

On-device layout per core: 32 chunks of (128 rows x 512 cols).  TensorE
accumulates the 4 block-rows of the upper Gram in 4 PSUM banks with
fp32r (tf32) matmuls; ScalarE squares each chunk (accumulating per-row
||x||^2); VectorE squares x^2 with an accumulating reduce for sum x^4.
"""

import numpy as np

import concourse.bacc as bacc
import concourse.bass as bass
import concourse.mybir as mybir
import concourse.tile as tile
from concourse.bass_utils import run_bass_kernel_spmd

B, L, D = 8, 4096, 512
P = 128
NT = L // P  # 32 row-chunks per core
QC = 4       # chunks per DMA/compute quad
NCORES = 8
N_TOTAL = B * L

# Column start of the moving operand for block-row m.  Block-row m covers
# G rows [128m, 128m+128).  The upper triangle needs cols >= 128m; m=3 is
# widened to 256 cols because fp32r matmuls below 256 moving columns run
# at 1/4 rate (cost model) -- the extra block is discarded on the host.
COL0 = (0, 128, 256, 256)

_NC_CACHE = {}


def _build_nc():
    f32 = mybir.dt.float32
    f32r = mybir.dt.float32r
    Act = mybir.ActivationFunctionType
    Alu = mybir.AluOpType

    nc = bacc.Bacc("TRN2", target_bir_lowering=False, debug=False,
                   num_devices=NCORES)
    # x is fed PRE-ROUNDED to tf32 (see _round_tf32) and declared fp32r
    # end-to-end so the BIR verifier accepts it as a matmul operand.
    x_d = nc.dram_tensor("x", [L, D], f32r, kind="ExternalInput").ap()
    g_d = nc.dram_tensor("g", [D, D], f32, kind="ExternalOutput").ap()
    s_d = nc.dram_tensor("s", [P, 2], f32, kind="ExternalOutput").ap()

    bf16 = mybir.dt.bfloat16
    NQ = NT // QC  # 8 quads of QC=4 chunks

    with tile.TileContext(nc) as tc:
        with (
            tc.tile_pool(name="xin", bufs=3) as xin_pool,
            tc.tile_pool(name="sq", bufs=2) as sq_pool,
            tc.tile_pool(name="scr", bufs=2) as scr_pool,
            tc.tile_pool(name="acc", bufs=1) as acc_pool,
            tc.tile_pool(name="gout", bufs=2) as gout_pool,
            tc.tile_pool(name="ps", bufs=1, space="PSUM") as ps_pool,
        ):
            rs = acc_pool.tile([P, NT], f32, tag="rs")    # ||x_n||^2 per row
            qt = acc_pool.tile([P, NQ], f32, tag="qt")    # sum x^4 per quad
            psum = [ps_pool.tile([P, D - COL0[m]], f32, tag=f"ps{m}",
                                 name=f"ps{m}")
                    for m in range(4)]

            for j in range(NQ):
                xq = xin_pool.tile([P, QC, D], f32r, tag="xq")
                src = x_d[j * QC * P:(j + 1) * QC * P, :].rearrange(
                    "(c p) d -> p c d", p=P)
                nc.sync.dma_start(xq[:], src)
                for c in range(QC):
                    for m in range(4):
                        nc.tensor.matmul(
                            psum[m][:],
                            xq[:, c, m * P:(m + 1) * P],
                            xq[:, c, COL0[m]:],
                            start=(j == 0 and c == 0),
                            stop=(j == NQ - 1 and c == QC - 1),
                        )
                # x2 in bf16: enough precision for the S2/S3 scalar sums.
                x2 = sq_pool.tile([P, QC, D], bf16, tag="x2")
                nc.scalar.activation(x2[:], xq[:].bitcast(f32), Act.Square)
                nc.vector.reduce_sum(rs[:, j * QC:(j + 1) * QC], x2[:],
                                     axis=mybir.AxisListType.X)
                # sum x^4: alternate the engine (60% ACT / 40% DVE) so both
                # stay under the DMA-paced quad budget.
                scr = scr_pool.tile([P, QC, D], bf16, tag="scr")
                if j % 5 < 3:
                    nc.scalar.activation(scr[:], x2[:], Act.Square,
                                         accum_out=qt[:, j:j + 1])
                else:
                    nc.vector.scalar_tensor_tensor(
                        out=scr[:], in0=x2[:], scalar=1.0, in1=x2[:],
                        op0=Alu.mult, op1=Alu.mult, accum_out=qt[:, j:j + 1])

            # Reduce the per-chunk partials: s_sb[:,0] = sum x^4,
            # s_sb[:,1] = sum (||x_n||^2)^2 (both per partition).
            s_sb = acc_pool.tile([P, 2], f32, tag="s_sb")
            rs2 = scr_pool.tile([P, NT], f32, tag="rs2")
            nc.vector.scalar_tensor_tensor(
                out=rs2[:], in0=rs[:], scalar=1.0, in1=rs[:],
                op0=Alu.mult, op1=Alu.mult, accum_out=s_sb[:, 1:2])
            nc.vector.reduce_sum(s_sb[:, 0:1], qt[:], axis=mybir.AxisListType.X)
            nc.sync.dma_start(s_d[:], s_sb[:])

            for m in range(4):
                gtile = gout_pool.tile([P, D - COL0[m]], f32, tag="gt")
                # Split the PSUM->SBUF copies across DVE and ACT so the
                # output tail isn't serialized behind one engine's queue.
                if m % 2 == 0:
                    nc.vector.tensor_copy(gtile[:], psum[m][:])
                else:
                    nc.scalar.copy(gtile[:], psum[m][:])
                nc.sync.dma_start(g_d[m * P:(m + 1) * P, COL0[m]:], gtile[:])

    nc.compile()
    return nc


def _get_nc():
    if "nc" not in _NC_CACHE:
        _NC_CACHE["nc"] = _build_nc()
    return _NC_CACHE["nc"]


def _run_device(x, trace=False, trace_cores=None):
    """x: (8, 4096, 512) fp32.  Returns BassKernelResults.

    x is declared fp32r device-side; measured HW fp32r matmul precision
    on raw fp32 inputs is ~4e-5 relmax (better than tf32 rounding), so
    no host-side pre-rounding is applied."""
    nc = _get_nc()
    in_maps = [{"x": np.ascontiguousarray(x[c])} for c in range(NCORES)]
    kwargs = {}
    if trace:
        kwargs["trace_cores"] = (trace_cores if trace_cores is not None
                                 else list(range(NCORES)))
    res = run_bass_kernel_spmd(nc, in_maps, list(range(NCORES)), trace=trace,
                               **kwargs)
    return res


def _postprocess(results, kappa):
    # All-reduce partials in float64.
    g_sum = np.zeros((D, D), np.float64)
    s_sum = np.zeros((P, 2), np.float64)
    for r in results:
        g_sum += r["g"]
        s_sum += r["s"]
    # Mirror the block-upper triangle (diagonal blocks are full + symmetric;
    # m=3's widened block and anything below the diagonal is dropped).
    g_full = np.triu(g_sum) + np.triu(g_sum, 1).T

    n = float(N_TOTAL)
    inv_d2 = 1.0 / (D * D)
    s1 = np.trace(g_full)
    s2 = float(s_sum[:, 0].sum())
    s3 = float(s_sum[:, 1].sum())

    corr = (s3 - s2) / n * inv_d2
    whit = (s2 - 2.0 * s1) / n * inv_d2 + D * inv_d2

    kap = float(kappa)
    g_mean = g_full / n
    diag = np.diag(g_mean).copy()
    grad = (1.0 - kap) * g_mean
    np.fill_diagonal(grad, kap * (diag - 1.0))

    return (grad.astype(np.float32), np.float32(corr), np.float32(whit))


def kernel(x, kappa):
    x = np.asarray(x, dtype=np.float32)
    assert x.shape == (B, L, D), x.shape
    res = _run_device(x)
    return _postprocess(res.results, kappa)


# revision 17
# speedup vs baseline: 1.0669x; 1.0669x over previous
"""DecorrLoss kernel for 8 trn2 NeuronCores.

Math (matches reference DecorrLoss.forward):
  x: (8, 4096, 512) fp32, flattened to N=32768 samples of d=512.
  G  = X^T X            (512 x 512 Gram, summed over all samples)
  S1 = sum x^2  = trace(G)
  S2 = sum x^4
  S3 = sum_n (||x_n||^2)^2
  grad             = (1-kappa) * offdiag(G/N) + kappa * (diag(G/N) - I)
  correlation_loss = (S3 - S2) / (N d^2)
  whitening_loss   = (S2 - 2 S1 + N d) / (N d^2)

Sharding: data-parallel over the batch axis -- core c processes x[c]
(4096 x 512), producing a partial Gram (block-upper triangle only; the
host mirrors it) and per-partition partial sums for S2/S3.  The host
all-reduces the 8 partials in float64 and applies the kappa formula.

On-device layout per core: 32 chunks of (128 rows x 512 cols).  TensorE
accumulates the 4 block-rows of the upper Gram in 4 PSUM banks with
fp32r (tf32) matmuls; ScalarE squares each chunk (accumulating per-row
||x||^2); VectorE squares x^2 with an accumulating reduce for sum x^4.
"""

import numpy as np

import concourse.bacc as bacc
import concourse.bass as bass
import concourse.mybir as mybir
import concourse.tile as tile
from concourse.bass_utils import run_bass_kernel_spmd

B, L, D = 8, 4096, 512
P = 128
NT = L // P  # 32 row-chunks per core
QC = 4       # chunks per DMA/compute quad
NCORES = 8
N_TOTAL = B * L

# Column start of the moving operand for block-row m.  Block-row m covers
# G rows [128m, 128m+128).  The upper triangle needs cols >= 128m; m=3 is
# widened to 256 cols because fp32r matmuls below 256 moving columns run
# at 1/4 rate (cost model) -- the extra block is discarded on the host.
COL0 = (0, 128, 256, 256)

_NC_CACHE = {}


def _build_nc():
    f32 = mybir.dt.float32
    f32r = mybir.dt.float32r
    Act = mybir.ActivationFunctionType
    Alu = mybir.AluOpType

    nc = bacc.Bacc("TRN2", target_bir_lowering=False, debug=False,
                   num_devices=NCORES)
    # x is declared fp32r end-to-end so the BIR verifier accepts it as a
    # matmul operand; raw fp32 bits are fed (measured HW precision ~4e-5).
    x_d = nc.dram_tensor("x", [L, D], f32r, kind="ExternalInput").ap()
    g_d = nc.dram_tensor("g", [D, D], f32, kind="ExternalOutput").ap()
    s_d = nc.dram_tensor("s", [P, 2], f32, kind="ExternalOutput").ap()

    bf16 = mybir.dt.bfloat16
    NQ = NT // QC  # 8 quads of QC=4 chunks

    with tile.TileContext(nc) as tc:
        with (
            tc.tile_pool(name="xin", bufs=4) as xin_pool,
            tc.tile_pool(name="sq", bufs=4) as sq_pool,
            tc.tile_pool(name="scr", bufs=3) as scr_pool,
            tc.tile_pool(name="acc", bufs=1) as acc_pool,
            tc.tile_pool(name="gout", bufs=2) as gout_pool,
            tc.tile_pool(name="ps", bufs=1, space="PSUM") as ps_pool,
        ):
            rs = acc_pool.tile([P, NT], f32, tag="rs")    # ||x_n||^2 per row
            qt = acc_pool.tile([P, NQ], f32, tag="qt")    # sum x^4 per quad
            psum = [ps_pool.tile([P, D - COL0[m]], f32, tag=f"ps{m}",
                                 name=f"ps{m}")
                    for m in range(4)]

            for j in range(NQ):
                xq = xin_pool.tile([P, QC, D], f32r, tag="xq")
                half = QC // 2
                for h in range(2):
                    rows = x_d[(j * QC + h * half) * P:
                               (j * QC + (h + 1) * half) * P, :]
                    nc.sync.dma_start(
                        xq[:, h * half:(h + 1) * half, :],
                        rows.rearrange("(c p) d -> p c d", p=P))
                for c in range(QC):
                    for m in range(4):
                        nc.tensor.matmul(
                            psum[m][:],
                            xq[:, c, m * P:(m + 1) * P],
                            xq[:, c, COL0[m]:],
                            start=(j == 0 and c == 0),
                            stop=(j == NQ - 1 and c == QC - 1),
                        )
                # x2 in bf16: enough precision for the S2/S3 scalar sums.
                x2 = sq_pool.tile([P, QC, D], bf16, tag="x2")
                nc.scalar.activation(x2[:], xq[:].bitcast(f32), Act.Square)
                nc.vector.reduce_sum(rs[:, j * QC:(j + 1) * QC], x2[:],
                                     axis=mybir.AxisListType.X)
                # sum x^4: alternate the engine (50/50 ACT/DVE) so both
                # stay under the DMA-paced quad budget.
                scr = scr_pool.tile([P, QC, D], bf16, tag="scr")
                if j % 2 == 0:
                    nc.scalar.activation(scr[:], x2[:], Act.Square,
                                         accum_out=qt[:, j:j + 1])
                else:
                    nc.vector.scalar_tensor_tensor(
                        out=scr[:], in0=x2[:], scalar=1.0, in1=x2[:],
                        op0=Alu.mult, op1=Alu.mult, accum_out=qt[:, j:j + 1])

            # Reduce the per-chunk partials: s_sb[:,0] = sum x^4,
            # s_sb[:,1] = sum (||x_n||^2)^2 (both per partition).
            s_sb = acc_pool.tile([P, 2], f32, tag="s_sb")
            rs2 = scr_pool.tile([P, NT], f32, tag="rs2")
            nc.vector.scalar_tensor_tensor(
                out=rs2[:], in0=rs[:], scalar=1.0, in1=rs[:],
                op0=Alu.mult, op1=Alu.mult, accum_out=s_sb[:, 1:2])
            nc.vector.reduce_sum(s_sb[:, 0:1], qt[:], axis=mybir.AxisListType.X)
            nc.sync.dma_start(s_d[:], s_sb[:])

            for m in range(4):
                gtile = gout_pool.tile([P, D - COL0[m]], f32, tag="gt")
                # Split the PSUM->SBUF copies across DVE and ACT so the
                # output tail isn't serialized behind one engine's queue.
                if m % 2 == 0:
                    nc.vector.tensor_copy(gtile[:], psum[m][:])
                else:
                    nc.scalar.copy(gtile[:], psum[m][:])
                nc.sync.dma_start(g_d[m * P:(m + 1) * P, COL0[m]:], gtile[:])

    nc.compile()
    return nc


def _get_nc():
    if "nc" not in _NC_CACHE:
        _NC_CACHE["nc"] = _build_nc()
    return _NC_CACHE["nc"]


def _run_device(x, trace=False, trace_cores=None):
    """x: (8, 4096, 512) fp32.  Returns BassKernelResults.

    x is declared fp32r device-side; measured HW fp32r matmul precision
    on raw fp32 inputs is ~4e-5 relmax (better than tf32 rounding), so
    no host-side pre-rounding is applied."""
    nc = _get_nc()
    in_maps = [{"x": np.ascontiguousarray(x[c])} for c in range(NCORES)]
    kwargs = {}
    if trace:
        kwargs["trace_cores"] = (trace_cores if trace_cores is not None
                                 else list(range(NCORES)))
    res = run_bass_kernel_spmd(nc, in_maps, list(range(NCORES)), trace=trace,
                               **kwargs)
    return res


def _postprocess(results, kappa):
    # All-reduce partials in float64.
    g_sum = np.zeros((D, D), np.float64)
    s_sum = np.zeros((P, 2), np.float64)
    for r in results:
        g_sum += r["g"]
        s_sum += r["s"]
    # Mirror the block-upper triangle (diagonal blocks are full + symmetric;
    # m=3's widened block and anything below the diagonal is dropped).
    g_full = np.triu(g_sum) + np.triu(g_sum, 1).T

    n = float(N_TOTAL)
    inv_d2 = 1.0 / (D * D)
    s1 = np.trace(g_full)
    s2 = float(s_sum[:, 0].sum())
    s3 = float(s_sum[:, 1].sum())

    corr = (s3 - s2) / n * inv_d2
    whit = (s2 - 2.0 * s1) / n * inv_d2 + D * inv_d2

    kap = float(kappa)
    g_mean = g_full / n
    diag = np.diag(g_mean).copy()
    grad = (1.0 - kap) * g_mean
    np.fill_diagonal(grad, kap * (diag - 1.0))

    return (grad.astype(np.float32), np.float32(corr), np.float32(whit))


def kernel(x, kappa):
    x = np.asarray(x, dtype=np.float32)
    assert x.shape == (B, L, D), x.shape
    res = _run_device(x)
    return _postprocess(res.results, kappa)


# revision 19
# speedup vs baseline: 1.1171x; 1.0470x over previous
"""DecorrLoss kernel for 8 trn2 NeuronCores.

Math (matches reference DecorrLoss.forward):
  x: (8, 4096, 512) fp32, flattened to N=32768 samples of d=512.
  G  = X^T X            (512 x 512 Gram, summed over all samples)
  S1 = sum x^2  = trace(G)
  S2 = sum x^4
  S3 = sum_n (||x_n||^2)^2
  grad             = (1-kappa) * offdiag(G/N) + kappa * (diag(G/N) - I)
  correlation_loss = (S3 - S2) / (N d^2)
  whitening_loss   = (S2 - 2 S1 + N d) / (N d^2)

Sharding: data-parallel over the batch axis -- core c processes x[c]
(4096 x 512), producing a partial Gram (block-upper triangle only; the
host mirrors it) and per-partition partial sums for S2/S3.  The host
all-reduces the 8 partials in float64 and applies the kappa formula.

On-device layout per core: 32 chunks of (128 rows x 512 cols).  TensorE
accumulates the 4 block-rows of the upper Gram in 4 PSUM banks with
fp32r (tf32) matmuls; ScalarE squares each chunk (accumulating per-row
||x||^2); VectorE squares x^2 with an accumulating reduce for sum x^4.
"""

import numpy as np

import concourse.bacc as bacc
import concourse.bass as bass
import concourse.mybir as mybir
import concourse.tile as tile
from concourse.bass_utils import run_bass_kernel_spmd

B, L, D = 8, 4096, 512
P = 128
NT = L // P  # 32 row-chunks per core
QC = 4       # chunks per DMA/compute quad
NCORES = 8
N_TOTAL = B * L

# Column start of the moving operand for block-row m.  Block-row m covers
# G rows [128m, 128m+128).  The upper triangle needs cols >= 128m; m=3 is
# widened to 256 cols because fp32r matmuls below 256 moving columns run
# at 1/4 rate (cost model) -- the extra block is discarded on the host.
COL0 = (0, 128, 256, 256)

_NC_CACHE = {}


def _build_nc():
    f32 = mybir.dt.float32
    f32r = mybir.dt.float32r
    Act = mybir.ActivationFunctionType
    Alu = mybir.AluOpType

    nc = bacc.Bacc("TRN2", target_bir_lowering=False, debug=False,
                   num_devices=NCORES)
    # x is declared fp32r end-to-end so the BIR verifier accepts it as a
    # matmul operand; raw fp32 bits are fed (measured HW precision ~4e-5).
    x_d = nc.dram_tensor("x", [L, D], f32r, kind="ExternalInput").ap()
    g_d = nc.dram_tensor("g", [D, D], f32, kind="ExternalOutput").ap()
    s_d = nc.dram_tensor("s", [P, 2], f32, kind="ExternalOutput").ap()

    bf16 = mybir.dt.bfloat16
    NQ = NT // QC  # 8 quads of QC=4 chunks

    with tile.TileContext(nc) as tc:
        with (
            tc.tile_pool(name="xin", bufs=3) as xin_pool,
            tc.tile_pool(name="sq", bufs=3) as sq_pool,
            tc.tile_pool(name="scr", bufs=2) as scr_pool,
            tc.tile_pool(name="acc", bufs=1) as acc_pool,
            tc.tile_pool(name="gout", bufs=2) as gout_pool,
            tc.tile_pool(name="ps", bufs=1, space="PSUM") as ps_pool,
        ):
            rs = acc_pool.tile([P, NT], f32, tag="rs")    # ||x_n||^2 per row
            qt = acc_pool.tile([P, NQ], f32, tag="qt")    # sum x^4 per quad
            psum = [ps_pool.tile([P, D - COL0[m]], f32, tag=f"ps{m}",
                                 name=f"ps{m}")
                    for m in range(4)]

            for j in range(NQ):
                xq = xin_pool.tile([P, QC, D], f32r, tag="xq")
                if j == 0:
                    # chunk-granular first load so the PE starts ~2us sooner
                    # (a single 1MB DMA lands on one DGE queue)
                    for c in range(QC):
                        rows = x_d[c * P:(c + 1) * P, :]
                        nc.sync.dma_start(xq[:, c, :], rows)
                else:
                    src = x_d[j * QC * P:(j + 1) * QC * P, :].rearrange(
                        "(c p) d -> p c d", p=P)
                    nc.sync.dma_start(xq[:], src)
                for c in range(QC):
                    for m in range(4):
                        nc.tensor.matmul(
                            psum[m][:],
                            xq[:, c, m * P:(m + 1) * P],
                            xq[:, c, COL0[m]:],
                            start=(j == 0 and c == 0),
                            stop=(j == NQ - 1 and c == QC - 1),
                        )
                # x2 in bf16: enough precision for the S2/S3 scalar sums.
                x2 = sq_pool.tile([P, QC, D], bf16, tag="x2")
                nc.scalar.activation(x2[:], xq[:].bitcast(f32), Act.Square)
                nc.vector.reduce_sum(rs[:, j * QC:(j + 1) * QC], x2[:],
                                     axis=mybir.AxisListType.X)
                # sum x^4: alternate the engine (5/8 ACT, 3/8 DVE) so both
                # stay under the DMA-paced quad budget.
                scr = scr_pool.tile([P, QC, D], bf16, tag="scr")
                if j % 2 == 0 or j == 7:
                    nc.scalar.activation(scr[:], x2[:], Act.Square,
                                         accum_out=qt[:, j:j + 1])
                else:
                    nc.vector.scalar_tensor_tensor(
                        out=scr[:], in0=x2[:], scalar=1.0, in1=x2[:],
                        op0=Alu.mult, op1=Alu.mult, accum_out=qt[:, j:j + 1])

            # Reduce the per-chunk partials: s_sb[:,0] = sum x^4,
            # s_sb[:,1] = sum (||x_n||^2)^2 (both per partition).
            s_sb = acc_pool.tile([P, 2], f32, tag="s_sb")
            rs2 = scr_pool.tile([P, NT], f32, tag="rs2")
            nc.vector.scalar_tensor_tensor(
                out=rs2[:], in0=rs[:], scalar=1.0, in1=rs[:],
                op0=Alu.mult, op1=Alu.mult, accum_out=s_sb[:, 1:2])
            nc.vector.reduce_sum(s_sb[:, 0:1], qt[:], axis=mybir.AxisListType.X)
            nc.sync.dma_start(s_d[:], s_sb[:])

            for m in range(4):
                gtile = gout_pool.tile([P, D - COL0[m]], f32, tag="gt")
                # Split the PSUM->SBUF copies across DVE and ACT so the
                # output tail isn't serialized behind one engine's queue.
                if m % 2 == 0:
                    nc.vector.tensor_copy(gtile[:], psum[m][:])
                else:
                    nc.scalar.copy(gtile[:], psum[m][:])
                nc.sync.dma_start(g_d[m * P:(m + 1) * P, COL0[m]:], gtile[:])

    nc.compile()
    return nc


def _get_nc():
    if "nc" not in _NC_CACHE:
        _NC_CACHE["nc"] = _build_nc()
    return _NC_CACHE["nc"]


def _run_device(x, trace=False, trace_cores=None):
    """x: (8, 4096, 512) fp32.  Returns BassKernelResults.

    x is declared fp32r device-side; measured HW fp32r matmul precision
    on raw fp32 inputs is ~4e-5 relmax (better than tf32 rounding), so
    no host-side pre-rounding is applied."""
    nc = _get_nc()
    in_maps = [{"x": np.ascontiguousarray(x[c])} for c in range(NCORES)]
    kwargs = {}
    if trace:
        kwargs["trace_cores"] = (trace_cores if trace_cores is not None
                                 else list(range(NCORES)))
    res = run_bass_kernel_spmd(nc, in_maps, list(range(NCORES)), trace=trace,
                               **kwargs)
    return res


def _postprocess(results, kappa):
    # All-reduce partials in float64.
    g_sum = np.zeros((D, D), np.float64)
    s_sum = np.zeros((P, 2), np.float64)
    for r in results:
        g_sum += r["g"]
        s_sum += r["s"]
    # Mirror the block-upper triangle (diagonal blocks are full + symmetric;
    # m=3's widened block and anything below the diagonal is dropped).
    g_full = np.triu(g_sum) + np.triu(g_sum, 1).T

    n = float(N_TOTAL)
    inv_d2 = 1.0 / (D * D)
    s1 = np.trace(g_full)
    s2 = float(s_sum[:, 0].sum())
    s3 = float(s_sum[:, 1].sum())

    corr = (s3 - s2) / n * inv_d2
    whit = (s2 - 2.0 * s1) / n * inv_d2 + D * inv_d2

    kap = float(kappa)
    g_mean = g_full / n
    diag = np.diag(g_mean).copy()
    grad = (1.0 - kap) * g_mean
    np.fill_diagonal(grad, kap * (diag - 1.0))

    return (grad.astype(np.float32), np.float32(corr), np.float32(whit))


def kernel(x, kappa):
    x = np.asarray(x, dtype=np.float32)
    assert x.shape == (B, L, D), x.shape
    res = _run_device(x)
    return _postprocess(res.results, kappa)


# revision 20
# speedup vs baseline: 1.2060x; 1.0796x over previous
"""DecorrLoss kernel for 8 trn2 NeuronCores.

Math (matches reference DecorrLoss.forward):
  x: (8, 4096, 512) fp32, flattened to N=32768 samples of d=512.
  G  = X^T X            (512 x 512 Gram, summed over all samples)
  S1 = sum x^2  = trace(G)
  S2 = sum x^4
  S3 = sum_n (||x_n||^2)^2
  grad             = (1-kappa) * offdiag(G/N) + kappa * (diag(G/N) - I)
  correlation_loss = (S3 - S2) / (N d^2)
  whitening_loss   = (S2 - 2 S1 + N d) / (N d^2)

Sharding: data-parallel over the batch axis -- core c processes x[c]
(4096 x 512), producing a partial Gram (block-upper triangle only; the
host mirrors it) and per-partition partial sums for S2/S3.  The host
all-reduces the 8 partials in float64 and applies the kappa formula.

On-device layout per core: 32 chunks of (128 rows x 512 cols).  TensorE
accumulates the 4 block-rows of the upper Gram in 4 PSUM banks with
fp32r (tf32) matmuls; ScalarE squares each chunk (accumulating per-row
||x||^2); VectorE squares x^2 with an accumulating reduce for sum x^4.
"""

import numpy as np

import concourse.bacc as bacc
import concourse.bass as bass
import concourse.mybir as mybir
import concourse.tile as tile
from concourse.bass_utils import run_bass_kernel_spmd

B, L, D = 8, 4096, 512
P = 128
NT = L // P  # 32 row-chunks per core
QC = 4       # chunks per DMA/compute quad
NCORES = 8
N_TOTAL = B * L

# Column start of the moving operand for block-row m.  Block-row m covers
# G rows [128m, 128m+128).  The upper triangle needs cols >= 128m; m=3 is
# widened to 256 cols because fp32r matmuls below 256 moving columns run
# at 1/4 rate (cost model) -- the extra block is discarded on the host.
COL0 = (0, 128, 256, 256)

_NC_CACHE = {}


def _build_nc():
    f32 = mybir.dt.float32
    f32r = mybir.dt.float32r
    Act = mybir.ActivationFunctionType
    Alu = mybir.AluOpType

    nc = bacc.Bacc("TRN2", target_bir_lowering=False, debug=False,
                   num_devices=NCORES)
    # x is declared fp32r end-to-end so the BIR verifier accepts it as a
    # matmul operand; raw fp32 bits are fed (measured HW precision ~4e-5).
    x_d = nc.dram_tensor("x", [L, D], f32r, kind="ExternalInput").ap()
    g_d = nc.dram_tensor("g", [D, D], f32, kind="ExternalOutput").ap()
    s_d = nc.dram_tensor("s", [P, 2], f32, kind="ExternalOutput").ap()

    bf16 = mybir.dt.bfloat16
    NQ = NT // QC  # 8 quads of QC=4 chunks

    with tile.TileContext(nc) as tc:
        with (
            tc.tile_pool(name="xin", bufs=3) as xin_pool,
            tc.tile_pool(name="sq", bufs=2) as sq_pool,
            tc.tile_pool(name="scr", bufs=2) as scr_pool,
            tc.tile_pool(name="acc", bufs=1) as acc_pool,
            tc.tile_pool(name="gout", bufs=2) as gout_pool,
            tc.tile_pool(name="ps", bufs=1, space="PSUM") as ps_pool,
        ):
            rs = acc_pool.tile([P, NT], f32, tag="rs")    # ||x_n||^2 per row
            qt = acc_pool.tile([P, NQ], f32, tag="qt")    # sum x^4 per quad
            psum = [ps_pool.tile([P, D - COL0[m]], f32, tag=f"ps{m}",
                                 name=f"ps{m}")
                    for m in range(4)]

            for j in range(NQ):
                xq = xin_pool.tile([P, QC, D], f32r, tag="xq")
                src = x_d[j * QC * P:(j + 1) * QC * P, :].rearrange(
                    "(c p) d -> p c d", p=P)
                nc.sync.dma_start(xq[:], src)
                for c in range(QC):
                    for m in range(4):
                        nc.tensor.matmul(
                            psum[m][:],
                            xq[:, c, m * P:(m + 1) * P],
                            xq[:, c, COL0[m]:],
                            start=(j == 0 and c == 0),
                            stop=(j == NQ - 1 and c == QC - 1),
                        )
                # x2 in bf16: enough precision for the S2/S3 scalar sums.
                x2 = sq_pool.tile([P, QC, D], bf16, tag="x2")
                nc.scalar.activation(x2[:], xq[:].bitcast(f32), Act.Square)
                nc.vector.reduce_sum(rs[:, j * QC:(j + 1) * QC], x2[:],
                                     axis=mybir.AxisListType.X)
                # sum x^4: alternate the engine (6/8 ACT, 2/8 DVE; measured
                # fastest split) so both stay under the DMA-paced quad budget.
                scr = scr_pool.tile([P, QC, D], bf16, tag="scr")
                if j % 5 < 3:
                    nc.scalar.activation(scr[:], x2[:], Act.Square,
                                         accum_out=qt[:, j:j + 1])
                else:
                    nc.vector.scalar_tensor_tensor(
                        out=scr[:], in0=x2[:], scalar=1.0, in1=x2[:],
                        op0=Alu.mult, op1=Alu.mult, accum_out=qt[:, j:j + 1])

            # Reduce the per-chunk partials: s_sb[:,0] = sum x^4,
            # s_sb[:,1] = sum (||x_n||^2)^2 (both per partition).
            s_sb = acc_pool.tile([P, 2], f32, tag="s_sb")
            rs2 = scr_pool.tile([P, NT], f32, tag="rs2")
            nc.vector.scalar_tensor_tensor(
                out=rs2[:], in0=rs[:], scalar=1.0, in1=rs[:],
                op0=Alu.mult, op1=Alu.mult, accum_out=s_sb[:, 1:2])
            nc.vector.reduce_sum(s_sb[:, 0:1], qt[:], axis=mybir.AxisListType.X)
            nc.sync.dma_start(s_d[:], s_sb[:])

            for m in range(4):
                gtile = gout_pool.tile([P, D - COL0[m]], f32, tag="gt")
                # Split the PSUM->SBUF copies across DVE and ACT so the
                # output tail isn't serialized behind one engine's queue.
                if m % 2 == 0:
                    nc.vector.tensor_copy(gtile[:], psum[m][:])
                else:
                    nc.scalar.copy(gtile[:], psum[m][:])
                nc.sync.dma_start(g_d[m * P:(m + 1) * P, COL0[m]:], gtile[:])

    nc.compile()
    return nc


def _get_nc():
    if "nc" not in _NC_CACHE:
        _NC_CACHE["nc"] = _build_nc()
    return _NC_CACHE["nc"]


def _run_device(x, trace=False, trace_cores=None):
    """x: (8, 4096, 512) fp32.  Returns BassKernelResults.

    x is declared fp32r device-side; measured HW fp32r matmul precision
    on raw fp32 inputs is ~4e-5 relmax (better than tf32 rounding), so
    no host-side pre-rounding is applied."""
    nc = _get_nc()
    in_maps = [{"x": np.ascontiguousarray(x[c])} for c in range(NCORES)]
    kwargs = {}
    if trace:
        kwargs["trace_cores"] = (trace_cores if trace_cores is not None
                                 else list(range(NCORES)))
    res = run_bass_kernel_spmd(nc, in_maps, list(range(NCORES)), trace=trace,
                               **kwargs)
    return res


def _postprocess(results, kappa):
    # All-reduce partials in float64.
    g_sum = np.zeros((D, D), np.float64)
    s_sum = np.zeros((P, 2), np.float64)
    for r in results:
        g_sum += r["g"]
        s_sum += r["s"]
    # Mirror the block-upper triangle (diagonal blocks are full + symmetric;
    # m=3's widened block and anything below the diagonal is dropped).
    g_full = np.triu(g_sum) + np.triu(g_sum, 1).T

    n = float(N_TOTAL)
    inv_d2 = 1.0 / (D * D)
    s1 = np.trace(g_full)
    s2 = float(s_sum[:, 0].sum())
    s3 = float(s_sum[:, 1].sum())

    corr = (s3 - s2) / n * inv_d2
    whit = (s2 - 2.0 * s1) / n * inv_d2 + D * inv_d2

    kap = float(kappa)
    g_mean = g_full / n
    diag = np.diag(g_mean).copy()
    grad = (1.0 - kap) * g_mean
    np.fill_diagonal(grad, kap * (diag - 1.0))

    return (grad.astype(np.float32), np.float32(corr), np.float32(whit))


def kernel(x, kappa):
    x = np.asarray(x, dtype=np.float32)
    assert x.shape == (B, L, D), x.shape
    res = _run_device(x)
    return _postprocess(res.results, kappa)
